# revision 31
# baseline (speedup 1.0000x reference)
"""ExpertNet (moe_routing) Trainium2 Bass kernel.

Data-parallel over 8 NeuronCores: batch N=32768 split into 8 shards of 4096.
All parameters replicated. Inside each core the pipeline is:

  X^T --(PE)--> h^T --(PE)--> z^T --(PE)--> dist/q --(PE broadcasts)--> z*q
     --(PE, row-tiled K=64 duos)--> expert hidden --(ACT/DVE relu, bf16)-->
     --(PE, col-tiled 128x32 quads)--> q-weighted logits sum --> preds^T
     --(DVE fold + 32x32 transpose)--> preds --> DRAM

PE array tiling is the core trick: the TensorE runs as independent sub-
arrays, so small matmuls execute CONCURRENTLY when they target distinct
row/col groups:
  * expert-hidden (K=64): pairs at tile_position (0,0)/(64,0) -> 2x.
  * expert-combine (K=128, M=32): quads at col positions 0/32/64/96
    accumulating into one [128,NB] PSUM bank -> ~4x; the four 32-row
    partial sums are folded by DVE at the end (3 adds).
  * q->pair broadcasts (K=16 zero-padded to 64): duos at (0,0)/(64,0).
  * dist -> q is ONE [z;z^2] x [-2mu^T;1] matmul (bias 1+|mu|^2 folded
    into an ACT Reciprocal), emitted twice to col groups 0/64 so the two
    q copies land partition-aligned for the padded-K broadcasts.
  * b2 enters via one K-padded [64,32] matmul vs q^T that accumulates
    straight into pp[32:64] (start=True also zero-fills that slice).

Everything expert-side (z-weights, h, z*q, W1, expert hidden, W2) runs in
bf16 (same PE rate as f32r, half the DVE/SBUF cost, ~0.1% noise that
averages across the 16 experts); the encoder matmul and the q chain stay
float32r.  q normalization (1/sum_k q) is a final per-column scaling.
"""

import numpy as np

N, D, H_ENC, NZ, KE, H_EXP, C = 32768, 1024, 512, 64, 16, 256, 10
NCORES = 8
NS = N // NCORES          # samples per core
NB = 512                  # samples per block (matmul moving free dim)
NBLK = NS // NB
NPAIR = KE // 2           # expert pairs

_CACHE = {}
LAST_RESULTS = None


def _build(has_b1: bool, cfg: dict | None = None):
    defaults = dict(pbig=4, pmisc=1, pqb=2, ppred=1, hbufs=9, ehbufs=36,
                    zqbufs=18, xbufs=3, repeat=1, W=NB, ahead=2,
                    relu_act_mod=2)
    cfg = {**defaults, **(cfg or {})}
    import concourse.bacc as bacc
    import concourse.mybir as mybir
    from concourse import tile

    F32 = mybir.dt.float32
    F32R = mybir.dt.float32r
    BF16 = mybir.dt.bfloat16
    AF = mybir.ActivationFunctionType

    W = cfg["W"]
    nc = bacc.Bacc("TRN2", target_bir_lowering=False, debug=False,
                   num_devices=NCORES)

    # ---- I/O ----------------------------------------------------------
    XT = nc.dram_tensor("XT", [8, 128, NS], BF16, kind="ExternalInput")
    Wenc = nc.dram_tensor("Wenc", [128, 8 * H_ENC], BF16, kind="ExternalInput")
    Wz = nc.dram_tensor("Wz", [128, 4 * NZ], BF16, kind="ExternalInput")
    W1p = nc.dram_tensor("W1p", [128, NPAIR * H_EXP], BF16, kind="ExternalInput")
    W2cn = nc.dram_tensor("W2cn", [128, KE * 2 * 32], BF16, kind="ExternalInput")
    DMU = nc.dram_tensor("DMU", [128, 32], F32R, kind="ExternalInput")
    BDR = nc.dram_tensor("BDR", [128, 1], F32, kind="ExternalInput")
    O16P = nc.dram_tensor("O16P", [64, 1], F32R, kind="ExternalInput")
    ONES1_32 = nc.dram_tensor("ONES1_32", [1, 32], F32R, kind="ExternalInput")
    E2P = nc.dram_tensor("E2P", [128, NPAIR * 128], F32R, kind="ExternalInput")
    B2P64 = nc.dram_tensor("B2P64", [64, 32], F32R, kind="ExternalInput")
    BENC = nc.dram_tensor("BENC", [128, 4], F32, kind="ExternalInput")
    BZ = nc.dram_tensor("BZ", [NZ, 1], F32, kind="ExternalInput")
    if has_b1:
        B1C = nc.dram_tensor("B1C", [128, KE * 2], F32, kind="ExternalInput")
        E2SP = nc.dram_tensor("E2SP", [128, KE * 128], F32R, kind="ExternalInput")
    OUT = nc.dram_tensor("OUT", [NS, C], F32, kind="ExternalOutput")

    with tile.TileContext(nc) as tc, nc.allow_low_precision(
        reason="float32r/bf16 tiles feed the PE; net rounding is ~3e-3 relative"
    ):
        with (
            tc.tile_pool(name="wpool", bufs=1) as wp,
            tc.tile_pool(name="xpool", bufs=cfg["xbufs"]) as xp,
            tc.tile_pool(name="hpool", bufs=cfg["hbufs"]) as hp,
            tc.tile_pool(name="zpool", bufs=2) as zp,
            tc.tile_pool(name="qpool", bufs=2) as qp,
            tc.tile_pool(name="zqpool", bufs=cfg["zqbufs"]) as zqp,
            tc.tile_pool(name="ehpool", bufs=cfg["ehbufs"]) as ehp,
            tc.tile_pool(name="trpool", bufs=4) as trp,
            tc.tile_pool(name="pbig", bufs=cfg["pbig"], space="PSUM") as pbig,
            tc.tile_pool(name="pmisc", bufs=cfg["pmisc"], space="PSUM") as pmisc,
            tc.tile_pool(name="pqb", bufs=max(cfg["pqb"], 1), space="PSUM") as pqb,
            tc.tile_pool(name="ppred", bufs=cfg["ppred"], space="PSUM") as ppred,
        ):
            # ---- load weights once -----------------------------------
            def wload(dram, shape, dt):
                t = wp.tile(shape, dt, name=dram.name + "_sb")
                nc.sync.dma_start(t[:], dram[:])
                return t

            # front-critical weights first; the big expert weights are
            # deferred until block 0's X DMAs are queued.
            wenc = wp.tile([128, 8 * H_ENC], BF16, name="Wenc_sb")
            for dc in range(8):
                nc.sync.dma_start(wenc[:, dc * H_ENC:(dc + 1) * H_ENC],
                                  Wenc[:, dc * H_ENC:(dc + 1) * H_ENC])
            benc = wload(BENC, [128, 4], F32)
            wz = wload(Wz, [128, 4 * NZ], BF16)
            dmu = wload(DMU, [128, 32], F32R)
            bdr = wload(BDR, [128, 1], F32)
            o16p = wload(O16P, [64, 1], F32R)
            o132 = wload(ONES1_32, [1, 32], F32R)
            bz = wload(BZ, [NZ, 1], F32)
            e2p = wload(E2P, [128, NPAIR * 128], F32R)
            late = {"e2p": e2p}

            # persistent rotating q tiles (rows 0:16 and 64:80 hold q; the
            # zero rows make the K=16->64 padded matmuls exact).  ahead+1
            # tiles so front(i+ahead)'s rewrite never lands on the tile
            # back(i) still reads (same-tile rewrites serialize by program
            # order, and fronts are emitted before their block's back).
            NQ = cfg["ahead"] + 1
            qr2_tiles = []
            for i in range(NQ):
                t = wp.tile([128, NB], F32R, name=f"qr2_{i}")
                nc.vector.memzero(t[:])
                qr2_tiles.append(t)

            def load_late_weights():
                late["w1p"] = wload(W1p, [128, NPAIR * H_EXP], BF16)
                late["w2cn"] = wload(W2cn, [128, KE * 2 * 32], BF16)
                late["b2p64"] = wload(B2P64, [64, 32], F32R)
                if has_b1:
                    late["b1c"] = wload(B1C, [128, KE * 2], F32)
                    late["e2sp"] = wload(E2SP, [128, KE * 128], F32R)

            def front(ib):
                n0 = ib * NB
                xt = xp.tile([128, 8 * NB], BF16, tag="xt")
                for dc in range(8):
                    nc.sync.dma_start(
                        xt[:, dc * NB:dc * NB + W], XT[dc, :, n0:n0 + W]
                    )

                # encoder: hT[hc] = relu(Wenc^T X^T + b)   (f32r, bf16 out)
                hts = []
                for hc in range(4):
                    ph = pbig.tile([128, NB], F32, tag="pbig")
                    for dc in range(8):
                        nc.tensor.matmul(
                            ph[:, :W],
                            wenc[:, dc * H_ENC + hc * 128: dc * H_ENC + (hc + 1) * 128],
                            xt[:, dc * NB:dc * NB + W],
                            start=(dc == 0), stop=(dc == 7),
                        )
                    ht = hp.tile([128, NB], BF16, tag="ht")
                    nc.scalar.activation(ht[:, :W], ph[:, :W], AF.Relu,
                                         bias=benc[:, hc:hc + 1])
                    hts.append(ht)

                # z layer: zT = Wz^T hT + bz   (bf16 matmuls)
                pz = pmisc.tile([NZ, NB], F32, tag="pmisc")
                for hc in range(4):
                    nc.tensor.matmul(
                        pz[:, :W], wz[:, hc * NZ:(hc + 1) * NZ], hts[hc][:, :W],
                        start=(hc == 0), stop=(hc == 3),
                    )
                # zzsq rows 0:64 = z, rows 64:128 = z^2 (for the fused dist
                # matmul); zt2 = z duplicated (for the per-pair q fold).
                zzsq = zp.tile([128, NB], F32R, tag="zzsq")
                nc.scalar.activation(zzsq[0:NZ, :W], pz[:, :W], AF.Identity,
                                     bias=bz[:])
                zt2 = zp.tile([128, NB], F32R, tag="zt2")
                nc.scalar.activation(zt2[0:NZ, :W], pz[:, :W], AF.Identity,
                                     bias=bz[:])
                nc.scalar.activation(zt2[NZ:128, :W], pz[:, :W], AF.Identity,
                                     bias=bz[:])
                nc.vector.tensor_mul(zzsq[NZ:128, :W], zzsq[0:NZ, :W],
                                     zzsq[0:NZ, :W])

                # dist = -2 z.mu + |z|^2, emitted to col groups 0 and 64 so
                # both q copies are partition-aligned; q = 1/(dist + 1+|mu|^2)
                # via ACT Reciprocal with per-partition bias.
                pdx = pmisc.tile([32, NB], F32, tag="pmisc")
                nc.tensor.matmul(pdx[0:32, :W], dmu[:], zzsq[:, :W],
                                 start=True, stop=True)
                qr2 = qr2_tiles[ib % NQ]
                dtmp = zp.tile([KE, NB], F32, tag="dtmp")
                nc.vector.tensor_scalar_add(dtmp[0:KE, :W], pdx[0:KE, :W],
                                            bdr[0:KE, :])
                nc.vector.reciprocal(qr2[0:KE, :W], dtmp[0:KE, :W])
                nc.sync.dma_start(qr2[64:64 + KE, :W], qr2[0:KE, :W])

                # q-normalization chain (deferred scaling of preds^T):
                # qsum -> 1/qsum -> broadcast to 32 rows.
                pqs = pmisc.tile([1, NB], F32, tag="pmisc")
                nc.tensor.matmul(pqs[:, :W], o16p[:], qr2[0:64, :W],
                                 start=True, stop=True)
                rqs = qp.tile([1, NB], F32R, tag="rqs")
                nc.vector.reciprocal(rqs[:, :W], pqs[:, :W])
                prb = pmisc.tile([32, NB], F32, tag="pmisc")
                nc.tensor.matmul(prb[:, :W], o132[:], rqs[:, :W],
                                 start=True, stop=True)
                prb_sb = qp.tile([32, NB], F32R, tag="prb_sb")
                nc.scalar.activation(prb_sb[:, :W], prb[:, :W], AF.Copy)

                # q -> pair broadcasts (row-tiled duos) + z*q folds
                zqs = []
                if not has_b1:
                    for jj in range(0, NPAIR, 2):
                        qb_e = pqb.tile([128, NB], F32, tag="pqb")
                        nc.tensor.matmul(
                            qb_e[:, :W],
                            late["e2p"][0:64, jj * 128:(jj + 1) * 128],
                            qr2[0:64, :W], start=True, stop=True)
                        qb_o = pqb.tile([128, NB], F32, tag="pqb")
                        nc.tensor.matmul(
                            qb_o[:, :W],
                            late["e2p"][64:128, (jj + 1) * 128:(jj + 2) * 128],
                            qr2[64:128, :W], start=True, stop=True)
                        zq_e = zqp.tile([128, NB], BF16, tag="zq")
                        nc.vector.tensor_mul(zq_e[:, :W], zt2[:, :W], qb_e[:, :W])
                        zq_o = zqp.tile([128, NB], BF16, tag="zq")
                        nc.vector.tensor_mul(zq_o[:, :W], zt2[:, :W], qb_o[:, :W])
                        zqs += [zq_e, zq_o]
                else:
                    zq_sh = zqp.tile([128, NB], BF16, tag="zq")
                    nc.scalar.activation(zq_sh[:, :W], zt2[:, :W], AF.Copy)
                    zqs = [zq_sh] * NPAIR

                return dict(zqs=zqs, qr2=qr2, prb_sb=prb_sb, n0=n0)

            def back(st):
                zqs, qr2, prb_sb, n0 = (st["zqs"], st["qr2"], st["prb_sb"],
                                        st["n0"])
                ib = n0 // NB
                pp = ppred.tile([32, NB], F32, tag="ppred")
                # b2 term; ALSO zero-fills the slice exactly (start=True).
                nc.tensor.matmul(pp[:, :W], late["b2p64"][:],
                                 qr2[0:64, :W], start=True, stop=False,
                                 skip_group_check=True)

                # expert hidden: row-tiled K=64 duos, relu -> bf16
                ehs = []
                ci = 0
                for j in range(NPAIR):
                    zq = zqs[j]
                    if has_b1:
                        pqk_e = pqb.tile([128, NB], F32, tag="pqb")
                        nc.tensor.matmul(
                            pqk_e[:, :W],
                            late["e2sp"][0:64, (2 * j) * 128:(2 * j + 1) * 128],
                            qr2[0:64, :W], start=True, stop=True)
                        pqk_o = pqb.tile([128, NB], F32, tag="pqb")
                        nc.tensor.matmul(
                            pqk_o[:, :W],
                            late["e2sp"][64:128, (2 * j + 1) * 128:(2 * j + 2) * 128],
                            qr2[64:128, :W], start=True, stop=True)
                        pqks = [pqk_e, pqk_o]
                    for hc in range(2):
                        for half in range(2):
                            k = 2 * j + half
                            idx = k * 2 + hc
                            pe_ = pbig.tile([128, NB], F32, tag="pbig")
                            nc.tensor.matmul(
                                pe_[:, :W],
                                late["w1p"][64 * half:64 * (half + 1),
                                    j * H_EXP + hc * 128: j * H_EXP + (hc + 1) * 128],
                                zq[64 * half:64 * (half + 1), :W],
                                start=True, stop=True,
                                tile_position=(64 * half, 0),
                            )
                            eh = ehp.tile([128, NB], BF16, tag="eh")
                            if not has_b1:
                                if ci % cfg["relu_act_mod"] != 1:
                                    nc.scalar.activation(eh[:, :W], pe_[:, :W],
                                                         AF.Relu, bias=0.0)
                                else:
                                    nc.vector.tensor_scalar_max(
                                        eh[:, :W], pe_[:, :W], 0.0)
                            else:
                                ehf = ehp.tile([128, NB], F32R, tag="ehf")
                                nc.scalar.activation(
                                    ehf[:, :W], pe_[:, :W], AF.Relu,
                                    bias=late["b1c"][:, idx:idx + 1])
                                nc.vector.tensor_mul(eh[:, :W], ehf[:, :W],
                                                     pqks[half][:, :W])
                            ehs.append(eh)
                            ci += 1

                # combine: serial K=128, M=32 matmuls accumulating into pp
                ncomb = NPAIR * 4
                for j in range(NPAIR):
                    for t in range(4):
                        hc, half = t // 2, t % 2
                        gi = j * 4 + t
                        eh = ehs[j * 4 + 2 * hc + half]
                        nc.tensor.matmul(
                            pp[:, :W],
                            late["w2cn"][:, gi * 32:(gi + 1) * 32],
                            eh[:, :W],
                            start=False, stop=(j * 4 + t == ncomb - 1),
                            skip_group_check=True,
                        )

                # normalize, transpose preds^T -> preds, store
                ti = trp.tile([32, NB], F32, tag="ti")
                nc.scalar.activation(ti[:, :W], pp[:, :W], AF.Copy)
                nc.vector.tensor_mul(ti[:, :W], ti[:, :W], prb_sb[:, :W])
                tr = trp.tile([32, NB], F32, tag="tr")
                nc.vector.transpose(tr[:, :W], ti[:, :W])
                nc.sync.dma_start(
                    OUT[n0:n0 + W, :].rearrange("(b p) c -> p b c", p=32),
                    tr[:].rearrange("p (b v) -> p b v", v=32)[:, 0:W // 32, 0:C],
                )

            # software pipeline: fronts run `ahead` blocks before their
            # backs so the PE always has encoder work queued.
            A = cfg["ahead"]
            for _rep in range(cfg["repeat"]):
                sts = [front(0)]
                if _rep == 0 and "w1p" not in late:
                    load_late_weights()
                for ib in range(1, min(A, NBLK)):
                    sts.append(front(ib))
                for ib in range(NBLK):
                    if ib + A < NBLK:
                        sts.append(front(ib + A))
                    back(sts[ib])
                sts.clear()

    nc.compile()
    return nc


def _prep(inputs):
    import ml_dtypes
    BF = ml_dtypes.bfloat16
    f = lambda a: np.ascontiguousarray(np.asarray(a, dtype=np.float32))
    X, enc_W, enc_b = f(inputs["X"]), f(inputs["enc_W"]), f(inputs["enc_b"])
    z_W, z_b, mu = f(inputs["z_W"]), f(inputs["z_b"]), f(inputs["mu"])
    W1, b1, W2, b2 = f(inputs["W1"]), f(inputs["b1"]), f(inputs["W2"]), f(inputs["b2"])

    has_b1 = bool(np.any(b1))

    XT = np.ascontiguousarray(X.T)                       # [D, N]
    dmu = np.zeros((128, 32), np.float32)
    dmu[0:NZ, 0:KE] = -2.0 * mu.T
    dmu[NZ:128, 0:KE] = 1.0
    bdr = np.zeros((128, 1), np.float32)
    bdr[0:KE, 0] = 1.0 + (mu.astype(np.float64) ** 2).sum(axis=1)
    bdr[64:64 + KE, 0] = bdr[0:KE, 0]
    o16p = np.zeros((64, 1), np.float32)
    o16p[0:KE, 0] = 1.0
    com = {
        "Wenc": np.ascontiguousarray(
            enc_W.reshape(8, 128, H_ENC).transpose(1, 0, 2).reshape(128, 8 * H_ENC)
        ).astype(BF),
        "Wz": np.ascontiguousarray(
            z_W.reshape(4, 128, NZ).transpose(1, 0, 2).reshape(128, 4 * NZ)
        ).astype(BF),
        "DMU": dmu,
        "BDR": bdr,
        "O16P": o16p,
        "ONES1_32": np.ones((1, 32), np.float32),
        "BENC": np.ascontiguousarray(enc_b.reshape(4, 128).T),
        "BZ": z_b.reshape(NZ, 1).copy(),
    }
    w1p = np.zeros((128, NPAIR * H_EXP), np.float32)
    e2p = np.zeros((128, NPAIR * 128), np.float32)
    for j in range(NPAIR):
        w1p[0:64, j * H_EXP:(j + 1) * H_EXP] = W1[2 * j]
        w1p[64:128, j * H_EXP:(j + 1) * H_EXP] = W1[2 * j + 1]
        base = 0 if j % 2 == 0 else 64
        e2p[base + 2 * j, j * 128: j * 128 + 64] = 1.0
        e2p[base + 2 * j + 1, j * 128 + 64: j * 128 + 128] = 1.0
    com["W1p"], com["E2P"] = w1p.astype(BF), e2p

    w2cn = np.zeros((128, KE * 2 * 32), np.float32)
    for j in range(NPAIR):
        for t in range(4):
            hc, half = t // 2, t % 2
            k = 2 * j + half
            gi = j * 4 + t
            w2cn[:, gi * 32:gi * 32 + C] = W2[k][hc * 128:(hc + 1) * 128, :]
    com["W2cn"] = w2cn.astype(BF)

    b2p64 = np.zeros((64, 32), np.float32)
    b2p64[0:KE, 0:C] = b2
    com["B2P64"] = b2p64

    if has_b1:
        b1c = np.zeros((128, KE * 2), np.float32)
        e2sp = np.zeros((128, KE * 128), np.float32)
        for k in range(KE):
            for hc in range(2):
                b1c[:, k * 2 + hc] = b1[k, hc * 128:(hc + 1) * 128]
            base = 0 if k % 2 == 0 else 64
            e2sp[base + k, k * 128:(k + 1) * 128] = 1.0
        com["B1C"], com["E2SP"] = b1c, e2sp

    in_maps = []
    for c in range(NCORES):
        m = dict(com)
        shard = np.ascontiguousarray(XT[:, c * NS:(c + 1) * NS]).astype(BF)
        m["XT"] = shard.reshape(8, 128, NS)
        in_maps.append(m)
    return in_maps, has_b1


def kernel(**inputs) -> np.ndarray:
    global LAST_RESULTS
    from concourse.bass_utils import run_bass_kernel_spmd

    in_maps, has_b1 = _prep(inputs)
    if has_b1 not in _CACHE:
        _CACHE[has_b1] = _build(has_b1)
    nc = _CACHE[has_b1]

    res = run_bass_kernel_spmd(nc, in_maps, list(range(NCORES)))
    LAST_RESULTS = res
    out = np.concatenate([res.results[c]["OUT"] for c in range(NCORES)], axis=0)
    return np.ascontiguousarray(out, dtype=np.float32)
